# revision 1
# baseline (speedup 1.0000x reference)
"""Sliding-window causal GQA attention (B=2,T=2048,E=1024,H=16,HKV=8,D=64,
win=1024) on 8 TRN2 NeuronCores.

Sharding: token-parallel, zero collectives. 8 cores = (batch b in {0,1}) x
(512-token slice s in {0..3}). Each core recomputes k/v for its 1024-token
halo; every query attends exactly its in-window keys.

Layout strategy (per core):
  - host pre-transposes/casts: xT[f-major], W*T, rope tables (head-repeated),
    so the device never transposes x or weights.
  - projections run token-major (lhsT = xT chunk), rope+rmsnorm token-major
    on DVE, then PE transposes q-hat/k-hat into D-major tiles.
  - attention in scoresT layout [key, query]: sT = kT.T @ qT, exp on ACT with
    per-key-partition halo bias folded into the activation bias, band mask
    multiplied on the 8 partial tiles, y accumulated as v_ext.T @ eT with an
    ones column producing the softmax denominators for free (M=65).
  - normalization via K=1 broadcast matmul + DVE multiply; c_proj token-major.
"""

import os
import sys

for _p in ("/opt/trn_rl_repo", "/root/.axon_site/_ro/trn_rl_repo"):
    if os.path.isdir(_p) and _p not in sys.path:
        sys.path.append(_p)

import json as _json

import ml_dtypes
import numpy as np

import concourse.bass as bass
import concourse.mybir as mybir
import concourse.tile as tile
from concourse.bass_utils import run_bass_kernel_spmd
from concourse.masks import make_identity

BF16 = mybir.dt.bfloat16
F32 = mybir.dt.float32
AF = mybir.ActivationFunctionType

B, T, E = 2, 2048, 1024
H, HKV, D = 16, 8, 64
WIN = 1024
OWN = 512           # own tokens per core
KT = 1536           # halo + own keys per core
NCORES = 8
NEG_BIAS = -30000.0
# head order permutation: position p holds head PERM[p]; pairs (4m,4m+2),
# (4m+1,4m+3) share a 128-row tile so q-row parity matches kv-row parity.
PERM = [4*m + d for m in range(4) for d in (0, 2, 1, 3)]

# ---------------------------------------------------------------------------
# walrus workaround: this build rejects >1 sync wait per instruction.
# ---------------------------------------------------------------------------


def _install_patches():
    import concourse.tile as tile_mod
    import concourse.bass2jax as bass2jax_mod
    from concourse.vector_clock import ScopedClock, VectorClock
    from concourse.tile_scheduler import N_PROCS
    from concourse.bass_utils import compile_bir_kernel as _orig_compile

    if getattr(tile_mod, "_ant_drain_split", False):
        return

    def _drain_and_barrier_split(self, tick_clock, wait_clock):
        nc = self.nc
        gc = tick_clock.global_clock
        for p in range(N_PROCS):
            if gc[p] <= 0:
                continue
            vc = VectorClock([gc[q] if q == p else 0 for q in range(N_PROCS)])
            nop_inst = nc.sync.nop(nofuse=True, hint=f"tile_exit_wait_p{p}")
            wait_clock.add_sem_waits(nop_inst.ins, ScopedClock({None: vc}))
        nc.sync.drain()
        nc.all_engine_barrier()
        assert self.sems is not None
        popped = nc._tile_sem_poison_stack.pop()
        assert popped is self._sem_poison
        nc.clear_and_free_semaphores(list(self.sems.allocated().values()))
        nc.all_engine_barrier()

    tile_mod.TileContext._drain_and_barrier = _drain_and_barrier_split
    tile_mod._ant_drain_split = True

    def _split_multiwaits(bir_json: bytes) -> bytes:
        j = _json.loads(bir_json)
        for fn in j.get("functions", []):
            for blk in fn.get("blocks", []):
                out = []
                for inst in blk.get("instructions", []):
                    si = inst.get("sync_info")
                    waits = (si or {}).get("on_wait", [])
                    if len(waits) > 1:
                        for k, w in enumerate(waits[:-1]):
                            nop = {
                                "engine": inst.get("engine", "SP"),
                                "ins": [],
                                "outs": [],
                                "name": f"{inst.get('name', 'I')}-ws{k}",
                                "opcode": "NoOp",
                                "sync_info": {"on_update": [], "on_wait": [w]},
                            }
                            if "queue" in inst:
                                nop["queue"] = inst["queue"]
                            out.append(nop)
                        si["on_wait"] = [waits[-1]]
                    out.append(inst)
                blk["instructions"] = out
        return _json.dumps(j).encode()

    def _patched_compile(bir_json, tmpdir, neff_name="file.neff"):
        return _orig_compile(_split_multiwaits(bir_json), tmpdir, neff_name)

    bass2jax_mod.compile_bir_kernel = _patched_compile


_install_patches()

# ---------------------------------------------------------------------------
# device kernel
# ---------------------------------------------------------------------------

ECH = E // 128          # 8 E-chunks
NTT = KT // 128         # 12 key token tiles
NQT = OWN // 128        # 4 own token tiles


def _build_nc():
    nc = bass.Bass("TRN2", target_bir_lowering=False, debug=False,
                   num_devices=NCORES)
    dr = {}
    dr["xT"] = nc.dram_tensor("xT", [E, KT], BF16, kind="ExternalInput")
    dr["ve"] = nc.dram_tensor("ve", [KT, HKV * D], BF16, kind="ExternalInput")
    dr["cosk"] = nc.dram_tensor("cosk", [KT, HKV * 32], BF16, kind="ExternalInput")
    dr["sink"] = nc.dram_tensor("sink", [KT, HKV * 32], BF16, kind="ExternalInput")
    dr["cosq"] = nc.dram_tensor("cosq", [OWN, H * 32], BF16, kind="ExternalInput")
    dr["sinq"] = nc.dram_tensor("sinq", [OWN, H * 32], BF16, kind="ExternalInput")
    dr["hv"] = nc.dram_tensor("hv", [128, NTT], F32, kind="ExternalInput")
    dr["maskT"] = nc.dram_tensor("maskT", [8, 128, OWN], BF16, kind="ExternalInput")
    dr["WqT"] = nc.dram_tensor("WqT", [E, H * D], BF16, kind="ExternalInput")
    dr["WkT"] = nc.dram_tensor("WkT", [E, HKV * D], BF16, kind="ExternalInput")
    dr["WvT"] = nc.dram_tensor("WvT", [E, HKV * D], BF16, kind="ExternalInput")
    dr["WgT"] = nc.dram_tensor("WgT", [32, HKV], BF16, kind="ExternalInput")
    dr["WpT"] = nc.dram_tensor("WpT", [E, E], BF16, kind="ExternalInput")
    dr["out"] = nc.dram_tensor("out", [OWN, E], F32, kind="ExternalOutput")

    with tile.TileContext(nc) as tc:
        _kernel_body(nc, tc, dr)
    return nc


def _kernel_body(nc, tc, dr):
    import contextlib
    ctx = contextlib.ExitStack()
    with ctx:
        persist = ctx.enter_context(tc.tile_pool(name="persist", bufs=1))
        # persistent SBUF tensors
        xT = [persist.tile([128, KT], BF16, tag=f"xT{c}", name=f"xT{c}") for c in range(ECH)]
        WqTs = [persist.tile([128, H * D], BF16, tag=f"wq{c}", name=f"wq{c}") for c in range(ECH)]
        WkTs = [persist.tile([128, HKV * D], BF16, tag=f"wk{c}", name=f"wk{c}") for c in range(ECH)]
        WvTs = [persist.tile([128, HKV * D], BF16, tag=f"wv{c}", name=f"wv{c}") for c in range(ECH)]
        WpTs = [persist.tile([128, E], BF16, tag=f"wp{c}", name=f"wp{c}") for c in range(ECH)]
        WgTt = persist.tile([32, HKV], BF16, tag="wg", name="wg")
        cosk = persist.tile([128, NTT * HKV * 32], BF16, tag="cosk", name="cosk")
        sink = persist.tile([128, NTT * HKV * 32], BF16, tag="sink", name="sink")
        cosq = persist.tile([128, NQT * H * 32], BF16, tag="cosq", name="cosq")
        sinq = persist.tile([128, NQT * H * 32], BF16, tag="sinq", name="sinq")
        hv = persist.tile([128, NTT], F32, tag="hv", name="hv")
        maskT = persist.tile([128, 8 * OWN], BF16, tag="maskT", name="maskT")
        ident = persist.tile([128, 128], BF16, tag="ident", name="ident")
        ones64 = persist.tile([1, 64], F32, tag="ones64", name="ones64")
        epsb = persist.tile([128, 1], F32, tag="epsb", name="epsb")
        # D-major q/k, v_ext, yT storage
        kTt = [persist.tile([128, KT], BF16, tag=f"kT{i}", name=f"kT{i}") for i in range(HKV * D // 128)]
        qTt = [persist.tile([128, OWN], BF16, tag=f"qT{i}", name=f"qT{i}") for i in range(H * D // 128)]
        vex = [persist.tile([128, HKV * 65], BF16, tag=f"vex{i}", name=f"vex{i}") for i in range(NTT)]
        yT = [persist.tile([128, OWN], BF16, tag=f"yT{i}", name=f"yT{i}") for i in range(H * D // 128)]
        gate = persist.tile([128, NTT * HKV], F32, tag="gate", name="gate")

        # loads
        for c in range(ECH):
            nc.sync.dma_start(xT[c][:], dr["xT"][c * 128:(c + 1) * 128, :])
            nc.sync.dma_start(WqTs[c][:], dr["WqT"][c * 128:(c + 1) * 128, :])
            nc.sync.dma_start(WkTs[c][:], dr["WkT"][c * 128:(c + 1) * 128, :])
            nc.sync.dma_start(WvTs[c][:], dr["WvT"][c * 128:(c + 1) * 128, :])
            nc.sync.dma_start(WpTs[c][:], dr["WpT"][c * 128:(c + 1) * 128, :])
        nc.sync.dma_start(WgTt[:], dr["WgT"][:])
        nc.sync.dma_start(hv[:], dr["hv"][:])
        for j in range(8):
            nc.sync.dma_start(maskT[:, j * OWN:(j + 1) * OWN], dr["maskT"][j])
        csk = dr["cosk"].rearrange("(n p) w -> n p w", p=128)
        ssk = dr["sink"].rearrange("(n p) w -> n p w", p=128)
        for t in range(NTT):
            nc.sync.dma_start(cosk[:, t * HKV * 32:(t + 1) * HKV * 32], csk[t])
            nc.sync.dma_start(sink[:, t * HKV * 32:(t + 1) * HKV * 32], ssk[t])
        csq = dr["cosq"].rearrange("(n p) w -> n p w", p=128)
        ssq = dr["sinq"].rearrange("(n p) w -> n p w", p=128)
        for t in range(NQT):
            nc.sync.dma_start(cosq[:, t * H * 32:(t + 1) * H * 32], csq[t])
            nc.sync.dma_start(sinq[:, t * H * 32:(t + 1) * H * 32], ssq[t])
        make_identity(nc, ident[:])
        nc.vector.memset(ones64[:], 1.0)
        nc.vector.memset(epsb[:], float(np.finfo(np.float32).eps))
        for i in range(NTT):
            nc.gpsimd.memset(vex[i][:], 1.0)

        # ---- phase A: projections + rope/rms + transposes + v build ----
        pA = ctx.enter_context(tc.tile_pool(name="pA", bufs=2))

        # gates for all tiles first (single bank, freed early)
        with tc.tile_pool(name="psG", bufs=1, space="PSUM") as psG:
            gps = psG.tile([128, NTT * HKV], F32)
            for t in range(NTT):
                nc.tensor.matmul(gps[:, t * HKV:(t + 1) * HKV],
                                 xT[0][0:32, t * 128:(t + 1) * 128],
                                 WgTt[:], start=True, stop=True)
            nc.scalar.activation(gate[:], gps[:], AF.Sigmoid)
            nc.vector.tensor_scalar_mul(gate[:], gate[:], 2.0)

        psA = ctx.enter_context(contextlib.ExitStack())
        psK = psA.enter_context(tc.tile_pool(name="psK", bufs=2, space="PSUM"))
        psQ = psA.enter_context(tc.tile_pool(name="psQ", bufs=1, space="PSUM"))
        psT = psA.enter_context(tc.tile_pool(name="psT", bufs=2, space="PSUM"))

        def rope_rms(ps, nh, cos_ap, sin_ap, dst):
            # token-major rope + rmsnorm: ps [128, nh*64] f32 psum
            w = nh * 32
            t1 = pA.tile([128, w], F32, tag="r_t1", name="r_t1")
            t2 = pA.tile([128, w], F32, tag="r_t2", name="r_t2")
            rot = pA.tile([128, nh * 64], F32, tag="r_rot", name="r_rot")
            x1 = ps[:].rearrange("p (h two d) -> p h two d", two=2, d=32)[:, :, 0, :]
            x2 = ps[:].rearrange("p (h two d) -> p h two d", two=2, d=32)[:, :, 1, :]
            r1 = rot[:].rearrange("p (h two d) -> p h two d", two=2, d=32)[:, :, 0, :]
            r2 = rot[:].rearrange("p (h two d) -> p h two d", two=2, d=32)[:, :, 1, :]
            cos3 = cos_ap.rearrange("p (h d) -> p h d", d=32)
            sin3 = sin_ap.rearrange("p (h d) -> p h d", d=32)
            t13 = t1[:].rearrange("p (h d) -> p h d", d=32)
            t23 = t2[:].rearrange("p (h d) -> p h d", d=32)
            nc.vector.tensor_mul(t13, x1, cos3)
            nc.vector.tensor_mul(t23, x2, sin3)
            nc.vector.tensor_add(r1, t13, t23)
            nc.vector.tensor_mul(t13, x2, cos3)
            nc.vector.tensor_mul(t23, x1, sin3)
            nc.vector.tensor_sub(r2, t13, t23)
            sq = pA.tile([128, nh * 64], F32, tag="r_sq", name="r_sq")
            nc.vector.tensor_mul(sq[:], rot[:], rot[:])
            ms = pA.tile([128, nh], F32, tag="r_ms", name="r_ms")
            nc.vector.tensor_reduce(ms[:], sq[:].rearrange("p (h d) -> p h d", d=64),
                                    axis=mybir.AxisListType.X, op=mybir.AluOpType.add)
            rr = pA.tile([128, nh], F32, tag="r_rr", name="r_rr")
            nc.scalar.activation(rr[:], ms[:], AF.Sqrt, bias=epsb[:], scale=1.0 / 64.0)
            nc.vector.reciprocal(rr[:], rr[:])
            for hh in range(nh):
                nc.vector.tensor_scalar_mul(dst[:, hh * 64:(hh + 1) * 64],
                                            rot[:, hh * 64:(hh + 1) * 64],
                                            rr[:, hh:hh + 1])

        for t in range(NTT):
            tsl = slice(t * 128, (t + 1) * 128)
            kp = psK.tile([128, HKV * D], F32, tag="kp", name="kp")
            vp = psK.tile([128, HKV * D], F32, tag="vp", name="vp")
            for c in range(ECH):
                nc.tensor.matmul(kp[:], xT[c][:, tsl], WkTs[c][:],
                                 start=(c == 0), stop=(c == ECH - 1))
                nc.tensor.matmul(vp[:], xT[c][:, tsl], WvTs[c][:],
                                 start=(c == 0), stop=(c == ECH - 1))
            # k-hat (token-major) then transpose to D-major
            kh = pA.tile([128, HKV * D], BF16, tag="kh", name="kh")
            rope_rms(kp, HKV,
                     cosk[:, t * HKV * 32:(t + 1) * HKV * 32],
                     sink[:, t * HKV * 32:(t + 1) * HKV * 32], kh)
            for p in range(HKV * D // 128):
                tp = psT.tile([128, 128], BF16, tag="tp", name="tp")
                nc.tensor.transpose(tp[:], kh[:, p * 128:(p + 1) * 128], ident[:])
                nc.vector.tensor_copy(kTt[p][:, tsl], tp[:])
            # v_ext build: (ve * gate) + v  into 65-strided bf16
            vet = pA.tile([128, HKV * D], BF16, tag="vet", name="vet")
            nc.sync.dma_start(vet[:], dr["ve"][tsl, :])
            for j in range(HKV):
                nc.vector.scalar_tensor_tensor(
                    out=vex[t][:, j * 65:j * 65 + 64],
                    in0=vet[:, j * 64:(j + 1) * 64],
                    scalar=gate[:, t * HKV + j:t * HKV + j + 1],
                    in1=vp[:, j * 64:(j + 1) * 64],
                    op0=mybir.AluOpType.mult, op1=mybir.AluOpType.add)
            if t >= 8:
                tq = t - 8
                qp = psQ.tile([128, H * D], F32, tag="qp", name="qp")
                for c in range(ECH):
                    nc.tensor.matmul(qp[:, 0:512], xT[c][:, tsl], WqTs[c][:, 0:512],
                                     start=(c == 0), stop=(c == ECH - 1))
                    nc.tensor.matmul(qp[:, 512:1024], xT[c][:, tsl],
                                     WqTs[c][:, 512:1024],
                                     start=(c == 0), stop=(c == ECH - 1))
                qh = pA.tile([128, H * D], BF16, tag="qh", name="qh")
                rope_rms(qp, H,
                         cosq[:, tq * H * 32:(tq + 1) * H * 32],
                         sinq[:, tq * H * 32:(tq + 1) * H * 32], qh)
                qsl = slice(tq * 128, (tq + 1) * 128)
                for p in range(H * D // 128):
                    tp = psT.tile([128, 128], BF16, tag="tp", name="tp")
                    nc.tensor.transpose(tp[:], qh[:, p * 128:(p + 1) * 128], ident[:])
                    nc.vector.tensor_copy(qTt[p][:, qsl], tp[:])

        psA.close()

        # ---- phase B: attention per q-head ----
        pB = ctx.enter_context(tc.tile_pool(name="pB", bufs=3))
        psB = ctx.enter_context(contextlib.ExitStack())
        psS = psB.enter_context(tc.tile_pool(name="psS", bufs=3, space="PSUM"))
        psY = psB.enter_context(tc.tile_pool(name="psY", bufs=2, space="PSUM"))
        psN = psB.enter_context(tc.tile_pool(name="psN", bufs=2, space="PSUM"))
        # partial (band-masked) k-tiles and their mask index
        partial = {0: 0, 1: 1, 2: 2, 3: 3, 8: 4, 9: 5, 10: 6, 11: 7}
        for pos in range(H):
            h = PERM[pos]
            kv = h // 2
            kT_tile = kTt[kv // 2]
            krow = slice((kv % 2) * 64, (kv % 2) * 64 + 64)
            qT_tile = qTt[pos // 2]
            qrow = slice((pos % 2) * 64, (pos % 2) * 64 + 64)
            yp = psY.tile([65, OWN], F32, tag="yp", name="yp")
            for j in range(NTT):
                sp = psS.tile([128, OWN], F32, tag="sp", name="sp")
                nc.tensor.matmul(sp[:], kT_tile[krow, j * 128:(j + 1) * 128],
                                 qT_tile[qrow, :], start=True, stop=True)
                eT = pB.tile([128, OWN], BF16, tag="eT", name="eT")
                nc.scalar.activation(eT[:], sp[:], AF.Exp,
                                     bias=hv[:, j:j + 1], scale=0.125)
                if j in partial:
                    m = partial[j]
                    nc.vector.tensor_mul(eT[:], eT[:],
                                         maskT[:, m * OWN:(m + 1) * OWN])
                nc.tensor.matmul(yp[:], vex[j][:, kv * 65:(kv + 1) * 65], eT[:],
                                 start=(j == 0), stop=(j == NTT - 1))
            rec = pB.tile([1, OWN], F32, tag="rec", name="rec")
            nc.vector.reciprocal(rec[:], yp[64:65, :])
            bc = psN.tile([64, OWN], F32, tag="bc", name="bc")
            nc.tensor.matmul(bc[:], ones64[:], rec[:], start=True, stop=True)
            bcs = pB.tile([64, OWN], F32, tag="bcs", name="bcs")
            nc.vector.tensor_copy(bcs[:], bc[:])
            dst = yT[pos // 2]
            if pos % 2 == 0:
                nc.vector.tensor_mul(dst[0:64, :], yp[0:64, :], bcs[:])
            else:
                ytmp = pB.tile([64, OWN], BF16, tag="ytmp", name="ytmp")
                nc.vector.tensor_mul(ytmp[:], yp[0:64, :], bcs[:])
                nc.sync.dma_start(dst[64:128, :], ytmp[:])

        psB.close()

        # ---- phase C: output projection (token-major) ----
        psO = ctx.enter_context(tc.tile_pool(name="psO", bufs=2, space="PSUM"))
        pC = ctx.enter_context(tc.tile_pool(name="pC", bufs=2))
        for m in range(NQT):
            msl = slice(m * 128, (m + 1) * 128)
            for eh in range(2):
                esl = slice(eh * 512, (eh + 1) * 512)
                op = psO.tile([128, 512], F32, tag="op", name="op")
                for c in range(ECH):
                    nc.tensor.matmul(op[:], yT[c][:, msl], WpTs[c][:, esl],
                                     start=(c == 0), stop=(c == ECH - 1))
                ot = pC.tile([128, 512], F32, tag="ot", name="ot")
                nc.vector.tensor_copy(ot[:], op[:])
                nc.sync.dma_start(dr["out"][msl, esl], ot[:])


# ---------------------------------------------------------------------------
# host side
# ---------------------------------------------------------------------------

_CACHE = {}


def _host_prep(x, ve, cos, sin, Wq, Wk, Wv, Wproj, Wgate):
    bf = ml_dtypes.bfloat16
    cos2 = np.asarray(cos, np.float32).reshape(T, 32)
    sin2 = np.asarray(sin, np.float32).reshape(T, 32)
    shared = {
        "WqT": np.ascontiguousarray(
            np.asarray(Wq, np.float32).T.reshape(E, H, D)[:, PERM, :]
            .reshape(E, H * D)).astype(bf),
        "WkT": np.ascontiguousarray(np.asarray(Wk, np.float32).T).astype(bf),
        "WvT": np.ascontiguousarray(np.asarray(Wv, np.float32).T).astype(bf),
        "WgT": np.ascontiguousarray(np.asarray(Wgate, np.float32).T).astype(bf),
        "WpT": np.ascontiguousarray(
            np.asarray(Wproj, np.float32).T.reshape(H, D, E)[PERM]
            .reshape(E, E)).astype(bf),
    }
    # band masks, scoresT layout, shared by all cores
    maskT = np.zeros((8, 128, OWN), np.float32)
    for m, j in enumerate([0, 1, 2, 3, 8, 9, 10, 11]):
        c = j * 128 + np.arange(128)[:, None]
        q = np.arange(OWN)[None, :]
        maskT[m] = ((c >= q + 1) & (c <= q + WIN)).astype(np.float32)
    shared["maskT"] = maskT.astype(bf)
    in_maps = []
    for core in range(NCORES):
        b, s = core // 4, core % 4
        base = s * 512
        lo = base - WIN
        n0 = max(0, -lo)
        xh = np.zeros((KT, E), np.float32)
        veh = np.zeros((KT, HKV * D), np.float32)
        ck = np.zeros((KT, 32), np.float32)
        sk = np.zeros((KT, 32), np.float32)
        xh[n0:] = x[b, lo + n0:base + OWN]
        veh[n0:] = ve[b, lo + n0:base + OWN]
        ck[n0:] = cos2[lo + n0:base + OWN]
        sk[n0:] = sin2[lo + n0:base + OWN]
        hvv = np.zeros((KT,), np.float32)
        hvv[:n0] = NEG_BIAS
        m = {
            "xT": np.ascontiguousarray(xh.T).astype(bf),
            "ve": veh.astype(bf),
            "cosk": np.tile(ck, (1, HKV)).astype(bf),
            "sink": np.tile(sk, (1, HKV)).astype(bf),
            "cosq": np.tile(ck[WIN:], (1, H)).astype(bf),
            "sinq": np.tile(sk[WIN:], (1, H)).astype(bf),
            "hv": np.ascontiguousarray(hvv.reshape(NTT, 128).T),
        }
        m.update(shared)
        in_maps.append(m)
    return in_maps


def _get_nc():
    if "nc" not in _CACHE:
        _CACHE["nc"] = _build_nc()
    return _CACHE["nc"]


def kernel(x, ve, cos, sin, Wq, Wk, Wv, Wproj, Wgate, window_size=WIN,
           _trace=False):
    assert int(window_size) == WIN
    in_maps = _host_prep(x, ve, cos, sin, Wq, Wk, Wv, Wproj, Wgate)
    nc = _get_nc()
    res = run_bass_kernel_spmd(nc, in_maps, core_ids=list(range(NCORES)),
                               trace=_trace)
    _CACHE["last_result"] = res
    out = np.zeros((B, T, E), np.float32)
    for core in range(NCORES):
        b, s = core // 4, core % 4
        out[b, s * 512:(s + 1) * 512] = res.results[core]["out"]
    return out



# revision 11
# speedup vs baseline: 1.1809x; 1.1809x over previous
"""Sliding-window causal GQA attention (B=2,T=2048,E=1024,H=16,HKV=8,D=64,
win=1024) on 8 TRN2 NeuronCores.

Sharding: token-parallel, zero collectives. 8 cores = (batch b in {0,1}) x
(512-token slice s in {0..3}). Each core recomputes k/v for its 1024-token
halo; every query attends exactly its in-window keys.

v2 design notes (vs the v1 baseline, 463us):
  - v1 ran the PE at the cold 1.2GHz HAM clock for the whole attention
    phase (276us of K=4/8): the per-head score->exp->mask->value chain
    starved the PE.  v2 restructures attention per *head pair* with the
    value matmuls staggered 3 j-tiles behind the score matmuls, exp work
    split between the ACT engine and a custom 8-stage DVE "exp ladder"
    op, band-masking moved to GpSimd, and softmax normalization via
    approximate reciprocal + GpSimd partition-broadcast.  The PE stream
    is dense, so the HAM clock gate stays warm (2.4GHz).
  - exp ladder: scores arrive pre-scaled by 1/256 (folded into the q
    rmsnorm), exp(s*0.125) ~= (f^2 + a f + b)^32 with f = s/256; the
    constant c^32 it is off by is applied to the ACT path via its exp
    bias so numerator/denominator stay consistent; it cancels in the
    softmax.  End-to-end extra error ~2e-3 (budget 2e-2).
  - halo masking: dead (zero-padded) keys have k_hat = 0 -> score 0, and
    a per-core valid column in vex (the 65th value-matmul column, which
    also produces the softmax denominator) zeroes their num/den
    contribution.  No per-tile exp bias needed.
"""

import os
import sys

for _p in ("/opt/trn_rl_repo", "/root/.axon_site/_ro/trn_rl_repo"):
    if os.path.isdir(_p) and _p not in sys.path:
        sys.path.append(_p)

import json as _json

import ml_dtypes
import numpy as np

import concourse.bass as bass
import concourse.mybir as mybir
import concourse.tile as tile
from concourse.bass_utils import run_bass_kernel_spmd
from concourse.masks import make_identity

BF16 = mybir.dt.bfloat16
F32 = mybir.dt.float32
AF = mybir.ActivationFunctionType

B, T, E = 2, 2048, 1024
H, HKV, D = 16, 8, 64
WIN = 1024
OWN = 512           # own tokens per core
KT = 1536           # halo + own keys per core
NCORES = 8
# head order permutation: position p holds head PERM[p]; pairs (4m,4m+2),
# (4m+1,4m+3) share a 128-row tile so q-row parity matches kv-row parity.
PERM = [4*m + d for m in range(4) for d in (0, 2, 1, 3)]

# Schraudolph exp on DVE: e^(32 s) ~= bitcast_f32(int32(s*A_SCH + B_SCH));
# scores arrive pre-scaled by 1/256 (folded into the q rmsnorm) so the
# natural-exp argument is s*32.  Error +-3% per element, ~1.4e-3 end to end.
LOG2E = 1.4426950408889634
A_SCH = float(2**23 * LOG2E * 32.0)
B_SCH = float(127 * 2**23 - 366393.0)
MAGIC_RECIP = 0x7EF127EA    # bit-trick reciprocal seed, +1 Newton step

# which j-tiles (of 12) use the DVE exp vs ACT exp
DVE_EXP_J = frozenset((1, 4, 7, 10))
# j-tiles whose scores need the causal band mask
MASKED_J = (0, 1, 2, 3, 8, 9, 10, 11)
MASK_IDX = {j: m for m, j in enumerate(MASKED_J)}
VSTAG = 3           # value matmuls trail scores by this many j-tiles

# ---------------------------------------------------------------------------
# walrus workaround: this build rejects >1 sync wait per instruction.
# ---------------------------------------------------------------------------


def _install_patches():
    import concourse.tile as tile_mod
    import concourse.bass2jax as bass2jax_mod
    from concourse.vector_clock import ScopedClock, VectorClock
    from concourse.tile_scheduler import N_PROCS
    from concourse.bass_utils import compile_bir_kernel as _orig_compile

    if getattr(tile_mod, "_ant_drain_split", False):
        return

    def _drain_and_barrier_split(self, tick_clock, wait_clock):
        nc = self.nc
        gc = tick_clock.global_clock
        for p in range(N_PROCS):
            if gc[p] <= 0:
                continue
            vc = VectorClock([gc[q] if q == p else 0 for q in range(N_PROCS)])
            nop_inst = nc.sync.nop(nofuse=True, hint=f"tile_exit_wait_p{p}")
            wait_clock.add_sem_waits(nop_inst.ins, ScopedClock({None: vc}))
        nc.sync.drain()
        nc.all_engine_barrier()
        assert self.sems is not None
        popped = nc._tile_sem_poison_stack.pop()
        assert popped is self._sem_poison
        nc.clear_and_free_semaphores(list(self.sems.allocated().values()))
        nc.all_engine_barrier()

    tile_mod.TileContext._drain_and_barrier = _drain_and_barrier_split
    tile_mod._ant_drain_split = True

    def _split_multiwaits(bir_json: bytes) -> bytes:
        j = _json.loads(bir_json)
        for fn in j.get("functions", []):
            for blk in fn.get("blocks", []):
                out = []
                for inst in blk.get("instructions", []):
                    si = inst.get("sync_info")
                    waits = (si or {}).get("on_wait", [])
                    if len(waits) > 1:
                        for k, w in enumerate(waits[:-1]):
                            nop = {
                                "engine": inst.get("engine", "SP"),
                                "ins": [],
                                "outs": [],
                                "name": f"{inst.get('name', 'I')}-ws{k}",
                                "opcode": "NoOp",
                                "sync_info": {"on_update": [], "on_wait": [w]},
                            }
                            if "queue" in inst:
                                nop["queue"] = inst["queue"]
                            out.append(nop)
                        si["on_wait"] = [waits[-1]]
                    out.append(inst)
                blk["instructions"] = out
        return _json.dumps(j).encode()

    def _patched_compile(bir_json, tmpdir, neff_name="file.neff"):
        return _orig_compile(_split_multiwaits(bir_json), tmpdir, neff_name)

    bass2jax_mod.compile_bir_kernel = _patched_compile


_install_patches()


# ---------------------------------------------------------------------------
# device kernel
# ---------------------------------------------------------------------------

ECH = E // 128          # 8 E-chunks
NTT = KT // 128         # 12 key token tiles
NQT = OWN // 128        # 4 own token tiles


def _build_nc():
    nc = bass.Bass("TRN2", target_bir_lowering=False, debug=False,
                   num_devices=NCORES)
    dr = {}
    dr["xT"] = nc.dram_tensor("xT", [E, KT], BF16, kind="ExternalInput")
    dr["ve"] = nc.dram_tensor("ve", [KT, HKV * D], BF16, kind="ExternalInput")
    # rope tables: [cos|sin] and [sin|cos] interleaved per head (h, 2, 32)
    dr["csk1"] = nc.dram_tensor("csk1", [KT, HKV * 64], BF16, kind="ExternalInput")
    dr["csk2"] = nc.dram_tensor("csk2", [KT, HKV * 64], BF16, kind="ExternalInput")
    dr["csq1"] = nc.dram_tensor("csq1", [OWN, H * 64], BF16, kind="ExternalInput")
    dr["csq2"] = nc.dram_tensor("csq2", [OWN, H * 64], BF16, kind="ExternalInput")
    dr["validT"] = nc.dram_tensor("validT", [128, NTT], BF16, kind="ExternalInput")
    dr["maskT"] = nc.dram_tensor("maskT", [8, 128, OWN], BF16, kind="ExternalInput")
    dr["WqT"] = nc.dram_tensor("WqT", [E, H * D], BF16, kind="ExternalInput")
    dr["WkT"] = nc.dram_tensor("WkT", [E, HKV * D], BF16, kind="ExternalInput")
    dr["WvT"] = nc.dram_tensor("WvT", [E, HKV * D], BF16, kind="ExternalInput")
    dr["WgT"] = nc.dram_tensor("WgT", [32, HKV], BF16, kind="ExternalInput")
    dr["WpT"] = nc.dram_tensor("WpT", [E, E], BF16, kind="ExternalInput")
    dr["out"] = nc.dram_tensor("out", [OWN, E], F32, kind="ExternalOutput")

    with tile.TileContext(nc) as tc:
        _kernel_body(nc, tc, dr)
    return nc


def _kernel_body(nc, tc, dr):
    import contextlib
    ctx = contextlib.ExitStack()
    with ctx:
        persist = ctx.enter_context(tc.tile_pool(name="persist", bufs=1))
        # phase-A-only pools (inputs + scratch), closed before phase B
        phA = contextlib.ExitStack()
        pin = phA.enter_context(tc.tile_pool(name="pin", bufs=1))
        xT = [pin.tile([128, KT], BF16, tag=f"xT{c}", name=f"xT{c}") for c in range(ECH)]
        WqTs = [pin.tile([128, H * D], BF16, tag=f"wq{c}", name=f"wq{c}") for c in range(ECH)]
        WkTs = [pin.tile([128, HKV * D], BF16, tag=f"wk{c}", name=f"wk{c}") for c in range(ECH)]
        WvTs = [pin.tile([128, HKV * D], BF16, tag=f"wv{c}", name=f"wv{c}") for c in range(ECH)]
        WpTs = [persist.tile([128, E], BF16, tag=f"wp{c}", name=f"wp{c}") for c in range(ECH)]
        WgTt = pin.tile([32, HKV], BF16, tag="wg", name="wg")
        csk1 = pin.tile([128, NTT * HKV * 64], BF16, tag="csk1", name="csk1")
        csk2 = pin.tile([128, NTT * HKV * 64], BF16, tag="csk2", name="csk2")
        csq1 = pin.tile([128, NQT * H * 64], BF16, tag="csq1", name="csq1")
        csq2 = pin.tile([128, NQT * H * 64], BF16, tag="csq2", name="csq2")
        validT = persist.tile([128, NTT], BF16, tag="validT", name="validT")
        maskT = persist.tile([128, 8 * OWN], BF16, tag="maskT", name="maskT")
        ident = persist.tile([128, 128], BF16, tag="ident", name="ident")
        epsb = persist.tile([128, 1], F32, tag="epsb", name="epsb")
        epsbq = persist.tile([128, 1], F32, tag="epsbq", name="epsbq")
        magicT = persist.tile([1, 2 * OWN], mybir.dt.int32, tag="magicT", name="magicT")
        # D-major q/k, v_ext (64 D cols + valid col per kv head), yT storage
        kTt = [persist.tile([128, KT], BF16, tag=f"kT{i}", name=f"kT{i}") for i in range(HKV * D // 128)]
        qTt = [persist.tile([128, OWN], BF16, tag=f"qT{i}", name=f"qT{i}") for i in range(H * D // 128)]
        vex = [persist.tile([128, HKV * 65], BF16, tag=f"vex{i}", name=f"vex{i}") for i in range(NTT)]
        yT = [persist.tile([128, OWN], BF16, tag=f"yT{i}", name=f"yT{i}") for i in range(H * D // 128)]
        gate = persist.tile([128, NTT * HKV], F32, tag="gate", name="gate")

        # loads (k/v/x first — phase A needs them first)
        for c in range(ECH):
            nc.sync.dma_start(xT[c][:], dr["xT"][c * 128:(c + 1) * 128, :])
            nc.sync.dma_start(WkTs[c][:], dr["WkT"][c * 128:(c + 1) * 128, :])
            nc.sync.dma_start(WvTs[c][:], dr["WvT"][c * 128:(c + 1) * 128, :])
        nc.sync.dma_start(WgTt[:], dr["WgT"][:])
        nc.sync.dma_start(validT[:], dr["validT"][:])
        ck1 = dr["csk1"].rearrange("(n p) w -> n p w", p=128)
        ck2 = dr["csk2"].rearrange("(n p) w -> n p w", p=128)
        for t in range(NTT):
            w = HKV * 64
            nc.sync.dma_start(csk1[:, t * w:(t + 1) * w], ck1[t])
            nc.sync.dma_start(csk2[:, t * w:(t + 1) * w], ck2[t])
        for c in range(ECH):
            nc.sync.dma_start(WqTs[c][:], dr["WqT"][c * 128:(c + 1) * 128, :])
        cq1 = dr["csq1"].rearrange("(n p) w -> n p w", p=128)
        cq2 = dr["csq2"].rearrange("(n p) w -> n p w", p=128)
        for t in range(NQT):
            w = H * 64
            nc.sync.dma_start(csq1[:, t * w:(t + 1) * w], cq1[t])
            nc.sync.dma_start(csq2[:, t * w:(t + 1) * w], cq2[t])
        for j in range(8):
            nc.sync.dma_start(maskT[:, j * OWN:(j + 1) * OWN], dr["maskT"][j])
        for c in range(ECH):
            nc.sync.dma_start(WpTs[c][:], dr["WpT"][c * 128:(c + 1) * 128, :])
        make_identity(nc, ident[:])
        eps = float(np.finfo(np.float32).eps)
        nc.vector.memset(epsb[:], eps)
        nc.vector.memset(epsbq[:], eps * 65536.0)
        nc.vector.memset(magicT[:], MAGIC_RECIP)

        # ---- phase A: projections + rope/rms + transposes + v build ----
        pA = phA.enter_context(tc.tile_pool(name="pA", bufs=2))

        # gates for all tiles first (single bank, freed early)
        with tc.tile_pool(name="psG", bufs=1, space="PSUM") as psG:
            gps = psG.tile([128, NTT * HKV], F32)
            for t in range(NTT):
                nc.tensor.matmul(gps[:, t * HKV:(t + 1) * HKV],
                                 xT[0][0:32, t * 128:(t + 1) * 128],
                                 WgTt[:], start=True, stop=True)
            nc.scalar.activation(gate[:], gps[:], AF.Sigmoid)
            nc.vector.tensor_scalar_mul(gate[:], gate[:], 2.0)

        psK = phA.enter_context(tc.tile_pool(name="psK", bufs=2, space="PSUM"))
        psQ = phA.enter_context(tc.tile_pool(name="psQ", bufs=1, space="PSUM"))
        psT = phA.enter_context(tc.tile_pool(name="psT", bufs=2, space="PSUM"))

        def rope_rms(ps, nh, cs1_ap, cs2_ap, dst, eps_ap, sqrt_scale):
            # token-major rope + rmsnorm on DVE; ps [128, nh*64] f32 psum.
            # rope: 2 full-width muls against [cos|sin], [sin|cos] tables,
            # then paired add/sub; rms: sq + per-head reduce + sqrt +
            # approx-recip, applied via a stride-0 broadcast multiply.
            w = nh * 64
            ta = pA.tile([128, w], F32, tag="r_ta", name="r_ta")
            tb = pA.tile([128, w], F32, tag="r_tb", name="r_tb")
            rot = pA.tile([128, w], F32, tag="r_rot", name="r_rot")
            nc.vector.tensor_mul(ta[:], ps[:], cs1_ap)
            nc.vector.tensor_mul(tb[:], ps[:], cs2_ap)
            ta3 = ta[:].rearrange("p (h two d) -> p h two d", two=2, d=32)
            tb3 = tb[:].rearrange("p (h two d) -> p h two d", two=2, d=32)
            r3 = rot[:].rearrange("p (h two d) -> p h two d", two=2, d=32)
            nc.vector.tensor_add(r3[:, :, 0, :], ta3[:, :, 0, :], ta3[:, :, 1, :])
            nc.vector.tensor_sub(r3[:, :, 1, :], tb3[:, :, 1, :], tb3[:, :, 0, :])
            nc.vector.tensor_mul(ta[:], rot[:], rot[:])
            ms = pA.tile([128, nh], F32, tag="r_ms", name="r_ms")
            nc.vector.tensor_reduce(ms[:], ta[:].rearrange("p (h d) -> p h d", d=64),
                                    axis=mybir.AxisListType.X, op=mybir.AluOpType.add)
            rr = pA.tile([128, nh], F32, tag="r_rr", name="r_rr")
            nc.scalar.activation(rr[:], ms[:], AF.Sqrt, bias=eps_ap, scale=sqrt_scale)
            nc.vector.reciprocal(rr[:], rr[:])
            rrb = rr[:][:, :, None].broadcast_to([128, nh, 64])
            nc.vector.tensor_mul(dst.rearrange("p (h d) -> p h d", d=64),
                                 rot[:].rearrange("p (h d) -> p h d", d=64), rrb)

        # staggered pipeline: projections for tile t, transposes for t-1
        kh_tiles = {}
        qh_tiles = {}

        def emit_proj(t):
            tsl = slice(t * 128, (t + 1) * 128)
            kp = psK.tile([128, HKV * D], F32, tag="kp", name="kp")
            vp = psK.tile([128, HKV * D], F32, tag="vp", name="vp")
            for c in range(ECH):
                nc.tensor.matmul(kp[:], xT[c][:, tsl], WkTs[c][:],
                                 start=(c == 0), stop=(c == ECH - 1))
                nc.tensor.matmul(vp[:], xT[c][:, tsl], WvTs[c][:],
                                 start=(c == 0), stop=(c == ECH - 1))
            kh = pA.tile([128, HKV * D], BF16, tag="kh", name="kh")
            rope_rms(kp, HKV,
                     csk1[:, t * HKV * 64:(t + 1) * HKV * 64],
                     csk2[:, t * HKV * 64:(t + 1) * HKV * 64],
                     kh[:], epsb[:], 1.0 / 64.0)
            kh_tiles[t] = kh
            # v_ext build: (ve * gate) + v  into 65-strided bf16 + valid col
            vet = pA.tile([128, HKV * D], BF16, tag="vet", name="vet")
            nc.sync.dma_start(vet[:], dr["ve"][tsl, :])
            g3 = gate[:, t * HKV:(t + 1) * HKV][:, :, None].broadcast_to([128, HKV, 64])
            vx3 = vex[t][:].rearrange("p (h d) -> p h d", d=65)
            nc.vector.tensor_mul(vx3[:, :, 0:64],
                                 vet[:].rearrange("p (h d) -> p h d", d=64), g3)
            nc.vector.tensor_add(vx3[:, :, 0:64], vx3[:, :, 0:64],
                                 vp[:].rearrange("p (h d) -> p h d", d=64))
            v3 = validT[:, t:t + 1][:, :, None].broadcast_to([128, HKV, 1])
            nc.vector.tensor_copy(vx3[:, :, 64:65], v3)
            if t >= 8:
                tq = t - 8
                qp = psQ.tile([128, H * D], F32, tag="qp", name="qp")
                for c in range(ECH):
                    nc.tensor.matmul(qp[:, 0:512], xT[c][:, tsl], WqTs[c][:, 0:512],
                                     start=(c == 0), stop=(c == ECH - 1))
                    nc.tensor.matmul(qp[:, 512:1024], xT[c][:, tsl],
                                     WqTs[c][:, 512:1024],
                                     start=(c == 0), stop=(c == ECH - 1))
                qh = pA.tile([128, H * D], BF16, tag="qh", name="qh")
                rope_rms(qp, H,
                         csq1[:, tq * H * 64:(tq + 1) * H * 64],
                         csq2[:, tq * H * 64:(tq + 1) * H * 64],
                         qh[:], epsbq[:], 1024.0)
                qh_tiles[tq] = qh

        def emit_transposes(t):
            tsl = slice(t * 128, (t + 1) * 128)
            kh = kh_tiles.pop(t)
            for p in range(0, HKV * D // 128, 2):
                tp = psT.tile([128, 256], BF16, tag="tp", name="tp")
                nc.tensor.transpose(tp[:, 0:128], kh[:, p * 128:(p + 1) * 128], ident[:])
                nc.tensor.transpose(tp[:, 128:256], kh[:, (p + 1) * 128:(p + 2) * 128], ident[:])
                nc.vector.tensor_copy(kTt[p][:, tsl], tp[:, 0:128])
                nc.vector.tensor_copy(kTt[p + 1][:, tsl], tp[:, 128:256])
            if t >= 8:
                tq = t - 8
                qh = qh_tiles.pop(tq)
                qsl = slice(tq * 128, (tq + 1) * 128)
                for p in range(0, H * D // 128, 2):
                    tp = psT.tile([128, 256], BF16, tag="tp", name="tp")
                    nc.tensor.transpose(tp[:, 0:128], qh[:, p * 128:(p + 1) * 128], ident[:])
                    nc.tensor.transpose(tp[:, 128:256], qh[:, (p + 1) * 128:(p + 2) * 128], ident[:])
                    nc.vector.tensor_copy(qTt[p][:, qsl], tp[:, 0:128])
                    nc.vector.tensor_copy(qTt[p + 1][:, qsl], tp[:, 128:256])

        for t in range(NTT):
            emit_proj(t)
            if t >= 1:
                emit_transposes(t - 1)
        emit_transposes(NTT - 1)

        phA.close()

        # ---- phase B: attention per head pair ----
        pB = ctx.enter_context(tc.tile_pool(name="pB", bufs=VSTAG + 1))
        pBs = ctx.enter_context(tc.tile_pool(name="pBs", bufs=2))
        psB = ctx.enter_context(contextlib.ExitStack())
        psS = psB.enter_context(tc.tile_pool(name="psS", bufs=2, space="PSUM"))
        psY = psB.enter_context(tc.tile_pool(name="psY", bufs=2, space="PSUM"))

        for pi in range(H // 2):
            posA, posB = 2 * pi, 2 * pi + 1
            hA, hB = PERM[posA], PERM[posB]
            kvA, kvB = hA // 2, hB // 2
            ktile = kTt[pi // 2]
            qtile = qTt[pi]
            ypAB = psY.tile([128, 2 * OWN], F32, tag="ypAB", name="ypAB")
            ypA = ypAB[:, 0:OWN]
            ypB = ypAB[:, OWN:2 * OWN]
            eTs = {}

            def emit_scores(j):
                jsl = slice(j * 128, (j + 1) * 128)
                sp = psS.tile([128, 2 * OWN], F32, tag="sp", name="sp")
                # two row-group-concurrent K=64 matmuls (tile_position
                # auto-derives (0,0) / (64,0) from the base partitions)
                nc.tensor.matmul(sp[:, 0:OWN], ktile[0:64, jsl],
                                 qtile[0:64, :], start=True, stop=True)
                nc.tensor.matmul(sp[:, OWN:2 * OWN], ktile[64:128, jsl],
                                 qtile[64:128, :], start=True, stop=True)
                eT = pB.tile([128, 2 * OWN], BF16, tag="eT", name="eT")
                if j in DVE_EXP_J:
                    ei = pB.tile([128, 2 * OWN], mybir.dt.int32, tag="ei",
                                 name="ei", bufs=2)
                    nc.vector.tensor_scalar(
                        out=ei[:], in0=sp[:], scalar1=A_SCH, scalar2=B_SCH,
                        op0=mybir.AluOpType.mult, op1=mybir.AluOpType.add)
                    nc.vector.tensor_copy(eT[:], ei[:].bitcast(F32))
                else:
                    nc.scalar.activation(eT[:], sp[:], AF.Exp, scale=32.0)
                if j in MASK_IDX:
                    m = MASK_IDX[j]
                    mb = maskT[:, m * OWN:(m + 1) * OWN][:, None, :].broadcast_to([128, 2, OWN])
                    e3 = eT[:].rearrange("p (two w) -> p two w", two=2)
                    nc.gpsimd.tensor_tensor(e3, e3, mb, op=mybir.AluOpType.mult)
                eTs[j] = eT

            def emit_values(j):
                eT = eTs.pop(j)
                nc.tensor.matmul(ypA[0:65, :], vex[j][:, kvA * 65:(kvA + 1) * 65],
                                 eT[:, 0:OWN], start=(j == 0), stop=(j == NTT - 1))
                nc.tensor.matmul(ypB[0:65, :], vex[j][:, kvB * 65:(kvB + 1) * 65],
                                 eT[:, OWN:2 * OWN], start=(j == 0), stop=(j == NTT - 1))

            for j in range(NTT):
                emit_scores(j)
                if j >= VSTAG:
                    emit_values(j - VSTAG)
            for j in range(NTT - VSTAG, NTT):
                emit_values(j)

            # normalization: bit-trick reciprocal (+1 Newton) of the den
            # row [1, 2*OWN] (both heads), then DMA log-doubling broadcast.
            den = ypAB[64:65, :]
            y0i = pBs.tile([1, 2 * OWN], mybir.dt.int32, tag="y0i", name="y0i")
            nc.vector.tensor_tensor(y0i[:], magicT[:], den.bitcast(mybir.dt.int32),
                                    op=mybir.AluOpType.subtract)
            tt = pBs.tile([1, 2 * OWN], F32, tag="tt", name="tt")
            nc.vector.tensor_mul(tt[:], den, y0i[:].bitcast(F32))
            nc.vector.tensor_scalar(out=tt[:], in0=tt[:], scalar1=-1.0,
                                    scalar2=2.0, op0=mybir.AluOpType.mult,
                                    op1=mybir.AluOpType.add)
            bcA = pBs.tile([64, OWN], F32, tag="bcA", name="bcA")
            bcB = pBs.tile([64, OWN], F32, tag="bcB", name="bcB")
            nc.vector.tensor_mul(bcA[0:1, :], y0i[:, 0:OWN].bitcast(F32), tt[:, 0:OWN])
            nc.vector.tensor_mul(bcB[0:1, :], y0i[:, OWN:2 * OWN].bitcast(F32), tt[:, OWN:2 * OWN])
            w = 1
            while w < 64:
                nc.sync.dma_start(bcA[w:2 * w, :], bcA[0:w, :])
                nc.sync.dma_start(bcB[w:2 * w, :], bcB[0:w, :])
                w *= 2
            dst = yT[pi]
            nc.vector.tensor_mul(dst[0:64, :], ypA[0:64, :], bcA[:])
            ytmp = pBs.tile([64, OWN], BF16, tag="ytmp", name="ytmp")
            nc.vector.tensor_mul(ytmp[:], ypB[0:64, :], bcB[:])
            nc.sync.dma_start(dst[64:128, :], ytmp[:])

        psB.close()

        # ---- phase C: output projection (token-major) ----
        psO = ctx.enter_context(tc.tile_pool(name="psO", bufs=2, space="PSUM"))
        pC = ctx.enter_context(tc.tile_pool(name="pC", bufs=2))
        for m in range(NQT):
            msl = slice(m * 128, (m + 1) * 128)
            for eh in range(2):
                esl = slice(eh * 512, (eh + 1) * 512)
                op = psO.tile([128, 512], F32, tag="op", name="op")
                for c in range(ECH):
                    nc.tensor.matmul(op[:], yT[c][:, msl], WpTs[c][:, esl],
                                     start=(c == 0), stop=(c == ECH - 1))
                ot = pC.tile([128, 512], F32, tag="ot", name="ot")
                nc.vector.tensor_copy(ot[:], op[:])
                nc.sync.dma_start(dr["out"][msl, esl], ot[:])


# ---------------------------------------------------------------------------
# host side
# ---------------------------------------------------------------------------

_CACHE = {}


def _host_prep(x, ve, cos, sin, Wq, Wk, Wv, Wproj, Wgate):
    bf = ml_dtypes.bfloat16
    cos2 = np.asarray(cos, np.float32).reshape(T, 32)
    sin2 = np.asarray(sin, np.float32).reshape(T, 32)
    shared = {
        "WqT": np.ascontiguousarray(
            np.asarray(Wq, np.float32).T.reshape(E, H, D)[:, PERM, :]
            .reshape(E, H * D)).astype(bf),
        "WkT": np.ascontiguousarray(np.asarray(Wk, np.float32).T).astype(bf),
        "WvT": np.ascontiguousarray(np.asarray(Wv, np.float32).T).astype(bf),
        "WgT": np.ascontiguousarray(np.asarray(Wgate, np.float32).T).astype(bf),
        "WpT": np.ascontiguousarray(
            np.asarray(Wproj, np.float32).T.reshape(H, D, E)[PERM]
            .reshape(E, E)).astype(bf),
    }
    # band masks, scoresT layout, shared by all cores
    maskT = np.zeros((8, 128, OWN), np.float32)
    for m, j in enumerate(MASKED_J):
        c = j * 128 + np.arange(128)[:, None]
        q = np.arange(OWN)[None, :]
        maskT[m] = ((c >= q + 1) & (c <= q + WIN)).astype(np.float32)
    shared["maskT"] = maskT.astype(bf)
    in_maps = []
    for core in range(NCORES):
        b, s = core // 4, core % 4
        base = s * 512
        lo = base - WIN
        n0 = max(0, -lo)
        xh = np.zeros((KT, E), np.float32)
        veh = np.zeros((KT, HKV * D), np.float32)
        ck = np.zeros((KT, 32), np.float32)
        sk = np.zeros((KT, 32), np.float32)
        xh[n0:] = x[b, lo + n0:base + OWN]
        veh[n0:] = ve[b, lo + n0:base + OWN]
        ck[n0:] = cos2[lo + n0:base + OWN]
        sk[n0:] = sin2[lo + n0:base + OWN]
        valid = np.zeros((KT,), np.float32)
        valid[n0:] = 1.0
        csk = np.concatenate([ck, sk], axis=1)       # [KT, 64] = [cos|sin]
        ssk = np.concatenate([sk, ck], axis=1)       # [KT, 64] = [sin|cos]
        m = {
            "xT": np.ascontiguousarray(xh.T).astype(bf),
            "ve": veh.astype(bf),
            "csk1": np.tile(csk, (1, HKV)).astype(bf),
            "csk2": np.tile(ssk, (1, HKV)).astype(bf),
            "csq1": np.tile(csk[WIN:], (1, H)).astype(bf),
            "csq2": np.tile(ssk[WIN:], (1, H)).astype(bf),
            "validT": np.ascontiguousarray(
                valid.reshape(NTT, 128).T).astype(bf),
        }
        m.update(shared)
        in_maps.append(m)
    return in_maps


def _get_nc():
    if "nc" not in _CACHE:
        _CACHE["nc"] = _build_nc()
    return _CACHE["nc"]


def kernel(x, ve, cos, sin, Wq, Wk, Wv, Wproj, Wgate, window_size=WIN,
           _trace=False):
    assert int(window_size) == WIN
    in_maps = _host_prep(x, ve, cos, sin, Wq, Wk, Wv, Wproj, Wgate)
    nc = _get_nc()
    res = run_bass_kernel_spmd(nc, in_maps, core_ids=list(range(NCORES)),
                               trace=_trace)
    _CACHE["last_result"] = res
    out = np.zeros((B, T, E), np.float32)
    for core in range(NCORES):
        b, s = core // 4, core % 4
        out[b, s * 512:(s + 1) * 512] = res.results[core]["out"]
    return out


# revision 16
# speedup vs baseline: 1.3807x; 1.1692x over previous
"""Sliding-window causal GQA attention (B=2,T=2048,E=1024,H=16,HKV=8,D=64,
win=1024) on 8 TRN2 NeuronCores.

Sharding: token-parallel, zero collectives. 8 cores = (batch b in {0,1}) x
(512-token slice s in {0..3}). Each core recomputes k/v for its 1024-token
halo; every query attends exactly its in-window keys.

v2 design notes (vs the v1 baseline, 463us):
  - v1 ran the PE at the cold 1.2GHz HAM clock for the whole attention
    phase (276us of K=4/8): the per-head score->exp->mask->value chain
    starved the PE.  v2 restructures attention per *head pair* with the
    value matmuls staggered 3 j-tiles behind the score matmuls, exp work
    split between the ACT engine and a custom 8-stage DVE "exp ladder"
    op, band-masking moved to GpSimd, and softmax normalization via
    approximate reciprocal + GpSimd partition-broadcast.  The PE stream
    is dense, so the HAM clock gate stays warm (2.4GHz).
  - exp ladder: scores arrive pre-scaled by 1/256 (folded into the q
    rmsnorm), exp(s*0.125) ~= (f^2 + a f + b)^32 with f = s/256; the
    constant c^32 it is off by is applied to the ACT path via its exp
    bias so numerator/denominator stay consistent; it cancels in the
    softmax.  End-to-end extra error ~2e-3 (budget 2e-2).
  - halo masking: dead (zero-padded) keys have k_hat = 0 -> score 0, and
    a per-core valid column in vex (the 65th value-matmul column, which
    also produces the softmax denominator) zeroes their num/den
    contribution.  No per-tile exp bias needed.
"""

import os
import sys

for _p in ("/opt/trn_rl_repo", "/root/.axon_site/_ro/trn_rl_repo"):
    if os.path.isdir(_p) and _p not in sys.path:
        sys.path.append(_p)

import json as _json

import ml_dtypes
import numpy as np

import concourse.bass as bass
import concourse.mybir as mybir
import concourse.tile as tile
from concourse.bass_utils import run_bass_kernel_spmd
from concourse.masks import make_identity

BF16 = mybir.dt.bfloat16
F32 = mybir.dt.float32
AF = mybir.ActivationFunctionType

B, T, E = 2, 2048, 1024
H, HKV, D = 16, 8, 64
WIN = 1024
OWN = 512           # own tokens per core
KT = 1536           # halo + own keys per core
NCORES = 8
# head order permutation: position p holds head PERM[p]; pairs (4m,4m+2),
# (4m+1,4m+3) share a 128-row tile so q-row parity matches kv-row parity.
PERM = [4*m + d for m in range(4) for d in (0, 2, 1, 3)]

# Schraudolph exp on DVE: e^(32 s) ~= bitcast_f32(int32(s*A_SCH + B_SCH));
# scores arrive pre-scaled by 1/256 (folded into the q rmsnorm) so the
# natural-exp argument is s*32.  Error +-3% per element, ~1.4e-3 end to end.
LOG2E = 1.4426950408889634
A_SCH = float(2**7 * LOG2E * 32.0)            # int16/bf16-bits variant
B_SCH = float(127 * 128 - 366393.0 / 65536.0)
MAGIC_RECIP = 0x7EF127EA    # bit-trick reciprocal seed, +1 Newton step

# which j-tiles (of 12) use the DVE exp vs ACT exp (keep unmasked tiles on
# DVE so the exp->mask->value chain stays short for masked ones)
DVE_EXP_J = frozenset((4, 5, 6))
# j-tiles whose scores need the causal band mask; mask multiply engine
# alternates between DVE and GpSimd to balance load
MASKED_J = (0, 1, 2, 3, 8, 9, 10, 11)
MASK_IDX = {j: m for m, j in enumerate(MASKED_J)}
GPS_MASK_J = frozenset((0, 2, 8, 10))
VSTAG = 5           # value matmuls trail scores by this many j-tiles

# ---------------------------------------------------------------------------
# walrus workaround: this build rejects >1 sync wait per instruction.
# ---------------------------------------------------------------------------


def _install_patches():
    import concourse.tile as tile_mod
    import concourse.bass2jax as bass2jax_mod
    from concourse.vector_clock import ScopedClock, VectorClock
    from concourse.tile_scheduler import N_PROCS
    from concourse.bass_utils import compile_bir_kernel as _orig_compile

    if getattr(tile_mod, "_ant_drain_split", False):
        return

    def _drain_and_barrier_split(self, tick_clock, wait_clock):
        nc = self.nc
        gc = tick_clock.global_clock
        for p in range(N_PROCS):
            if gc[p] <= 0:
                continue
            vc = VectorClock([gc[q] if q == p else 0 for q in range(N_PROCS)])
            nop_inst = nc.sync.nop(nofuse=True, hint=f"tile_exit_wait_p{p}")
            wait_clock.add_sem_waits(nop_inst.ins, ScopedClock({None: vc}))
        nc.sync.drain()
        nc.all_engine_barrier()
        assert self.sems is not None
        popped = nc._tile_sem_poison_stack.pop()
        assert popped is self._sem_poison
        nc.clear_and_free_semaphores(list(self.sems.allocated().values()))
        nc.all_engine_barrier()

    tile_mod.TileContext._drain_and_barrier = _drain_and_barrier_split
    tile_mod._ant_drain_split = True

    def _split_multiwaits(bir_json: bytes) -> bytes:
        j = _json.loads(bir_json)
        for fn in j.get("functions", []):
            for blk in fn.get("blocks", []):
                out = []
                for inst in blk.get("instructions", []):
                    si = inst.get("sync_info")
                    waits = (si or {}).get("on_wait", [])
                    if len(waits) > 1:
                        for k, w in enumerate(waits[:-1]):
                            nop = {
                                "engine": inst.get("engine", "SP"),
                                "ins": [],
                                "outs": [],
                                "name": f"{inst.get('name', 'I')}-ws{k}",
                                "opcode": "NoOp",
                                "sync_info": {"on_update": [], "on_wait": [w]},
                            }
                            if "queue" in inst:
                                nop["queue"] = inst["queue"]
                            out.append(nop)
                        si["on_wait"] = [waits[-1]]
                    out.append(inst)
                blk["instructions"] = out
        return _json.dumps(j).encode()

    def _patched_compile(bir_json, tmpdir, neff_name="file.neff"):
        return _orig_compile(_split_multiwaits(bir_json), tmpdir, neff_name)

    bass2jax_mod.compile_bir_kernel = _patched_compile


_install_patches()


# ---------------------------------------------------------------------------
# device kernel
# ---------------------------------------------------------------------------

ECH = E // 128          # 8 E-chunks
NTT = KT // 128         # 12 key token tiles
NQT = OWN // 128        # 4 own token tiles


def _build_nc():
    nc = bass.Bass("TRN2", target_bir_lowering=False, debug=False,
                   num_devices=NCORES)
    dr = {}
    dr["xT"] = nc.dram_tensor("xT", [E, KT], BF16, kind="ExternalInput")
    dr["ve"] = nc.dram_tensor("ve", [KT, HKV * D], BF16, kind="ExternalInput")
    # rope tables: [cos|sin] and [sin|cos] interleaved per head (h, 2, 32)
    dr["csk1"] = nc.dram_tensor("csk1", [KT, HKV * 64], BF16, kind="ExternalInput")
    dr["csk2"] = nc.dram_tensor("csk2", [KT, HKV * 64], BF16, kind="ExternalInput")
    dr["csq1"] = nc.dram_tensor("csq1", [OWN, H * 64], BF16, kind="ExternalInput")
    dr["csq2"] = nc.dram_tensor("csq2", [OWN, H * 64], BF16, kind="ExternalInput")
    dr["validT"] = nc.dram_tensor("validT", [128, NTT], BF16, kind="ExternalInput")
    dr["maskT"] = nc.dram_tensor("maskT", [8, 128, 2 * OWN], BF16, kind="ExternalInput")
    dr["WqT"] = nc.dram_tensor("WqT", [E, H * D], BF16, kind="ExternalInput")
    dr["WkT"] = nc.dram_tensor("WkT", [E, HKV * D], BF16, kind="ExternalInput")
    dr["WvT"] = nc.dram_tensor("WvT", [E, HKV * D], BF16, kind="ExternalInput")
    dr["WgT"] = nc.dram_tensor("WgT", [32, HKV], BF16, kind="ExternalInput")
    dr["WpT"] = nc.dram_tensor("WpT", [E, E], BF16, kind="ExternalInput")
    dr["out"] = nc.dram_tensor("out", [OWN, E], F32, kind="ExternalOutput")

    with tile.TileContext(nc) as tc:
        _kernel_body(nc, tc, dr)
    return nc


def _kernel_body(nc, tc, dr):
    import contextlib
    ctx = contextlib.ExitStack()
    with ctx:
        persist = ctx.enter_context(tc.tile_pool(name="persist", bufs=1))
        # phase-A-only pools (inputs + scratch), closed before phase B
        phA = contextlib.ExitStack()
        pin = phA.enter_context(tc.tile_pool(name="pin", bufs=1))
        xT = [pin.tile([128, KT], BF16, tag=f"xT{c}", name=f"xT{c}") for c in range(ECH)]
        WqTs = [pin.tile([128, H * D], BF16, tag=f"wq{c}", name=f"wq{c}") for c in range(ECH)]
        WkTs = [pin.tile([128, HKV * D], BF16, tag=f"wk{c}", name=f"wk{c}") for c in range(ECH)]
        WvTs = [pin.tile([128, HKV * D], BF16, tag=f"wv{c}", name=f"wv{c}") for c in range(ECH)]
        WpTs = [persist.tile([128, E], BF16, tag=f"wp{c}", name=f"wp{c}") for c in range(ECH)]
        WgTt = pin.tile([32, HKV], BF16, tag="wg", name="wg")
        csk1 = pin.tile([128, NTT * HKV * 64], BF16, tag="csk1", name="csk1")
        csk2 = pin.tile([128, NTT * HKV * 64], BF16, tag="csk2", name="csk2")
        csq1 = pin.tile([128, NQT * H * 64], BF16, tag="csq1", name="csq1")
        csq2 = pin.tile([128, NQT * H * 64], BF16, tag="csq2", name="csq2")
        validT = persist.tile([128, NTT], BF16, tag="validT", name="validT")
        maskT = persist.tile([128, 8 * 2 * OWN], BF16, tag="maskT", name="maskT")
        ident = persist.tile([128, 128], BF16, tag="ident", name="ident")
        epsb = persist.tile([128, 1], F32, tag="epsb", name="epsb")
        epsbq = persist.tile([128, 1], F32, tag="epsbq", name="epsbq")
        magicT = persist.tile([8, 128], mybir.dt.int32, tag="magicT", name="magicT")
        # D-major q/k, v_ext (64 D cols + valid col per kv head), yT storage
        kTt = [persist.tile([128, KT], BF16, tag=f"kT{i}", name=f"kT{i}") for i in range(HKV * D // 128)]
        qTt = [persist.tile([128, OWN], BF16, tag=f"qT{i}", name=f"qT{i}") for i in range(H * D // 128)]
        vex = [persist.tile([128, HKV * 65], BF16, tag=f"vex{i}", name=f"vex{i}") for i in range(NTT)]
        yT = [persist.tile([128, OWN], BF16, tag=f"yT{i}", name=f"yT{i}") for i in range(H * D // 128)]
        gate = persist.tile([128, NTT * HKV], F32, tag="gate", name="gate")

        # loads (k/v/x first — phase A needs them first)
        for c in range(ECH):
            nc.sync.dma_start(xT[c][:], dr["xT"][c * 128:(c + 1) * 128, :])
            nc.sync.dma_start(WkTs[c][:], dr["WkT"][c * 128:(c + 1) * 128, :])
            nc.sync.dma_start(WvTs[c][:], dr["WvT"][c * 128:(c + 1) * 128, :])
        nc.sync.dma_start(WgTt[:], dr["WgT"][:])
        nc.sync.dma_start(validT[:], dr["validT"][:])
        ck1 = dr["csk1"].rearrange("(n p) w -> n p w", p=128)
        ck2 = dr["csk2"].rearrange("(n p) w -> n p w", p=128)
        for t in range(NTT):
            w = HKV * 64
            nc.sync.dma_start(csk1[:, t * w:(t + 1) * w], ck1[t])
            nc.sync.dma_start(csk2[:, t * w:(t + 1) * w], ck2[t])
        for c in range(ECH):
            nc.sync.dma_start(WqTs[c][:], dr["WqT"][c * 128:(c + 1) * 128, :])
        cq1 = dr["csq1"].rearrange("(n p) w -> n p w", p=128)
        cq2 = dr["csq2"].rearrange("(n p) w -> n p w", p=128)
        for t in range(NQT):
            w = H * 64
            nc.sync.dma_start(csq1[:, t * w:(t + 1) * w], cq1[t])
            nc.sync.dma_start(csq2[:, t * w:(t + 1) * w], cq2[t])
        for j in range(8):
            nc.sync.dma_start(maskT[:, j * 2 * OWN:(j + 1) * 2 * OWN], dr["maskT"][j])
        for c in range(ECH):
            nc.sync.dma_start(WpTs[c][:], dr["WpT"][c * 128:(c + 1) * 128, :])
        make_identity(nc, ident[:])
        eps = float(np.finfo(np.float32).eps)
        nc.vector.memset(epsb[:], eps)
        nc.vector.memset(epsbq[:], eps * 65536.0)
        nc.vector.memset(magicT[:], MAGIC_RECIP)

        # ---- phase A: projections + rope/rms + transposes + v build ----
        pA = phA.enter_context(tc.tile_pool(name="pA", bufs=2))

        # gates for all tiles first (single bank, freed early)
        with tc.tile_pool(name="psG", bufs=1, space="PSUM") as psG:
            gps = psG.tile([128, NTT * HKV], F32)
            for t in range(NTT):
                nc.tensor.matmul(gps[:, t * HKV:(t + 1) * HKV],
                                 xT[0][0:32, t * 128:(t + 1) * 128],
                                 WgTt[:], start=True, stop=True)
            nc.scalar.activation(gate[:], gps[:], AF.Sigmoid)
            nc.vector.tensor_scalar_mul(gate[:], gate[:], 2.0)

        psK = phA.enter_context(tc.tile_pool(name="psK", bufs=2, space="PSUM"))
        psQ = phA.enter_context(tc.tile_pool(name="psQ", bufs=1, space="PSUM"))
        psT = phA.enter_context(tc.tile_pool(name="psT", bufs=2, space="PSUM"))

        def rope_rms(ps, nh, cs1_ap, cs2_ap, dst, eps_ap, sqrt_scale):
            # token-major rope + rmsnorm on DVE; ps [128, nh*64] f32 psum.
            # rope: 2 full-width muls against [cos|sin], [sin|cos] tables,
            # then paired add/sub; rms: sq + per-head reduce + sqrt +
            # approx-recip, applied via a stride-0 broadcast multiply.
            w = nh * 64
            ta = pA.tile([128, w], BF16, tag="r_ta", name="r_ta")
            tb = pA.tile([128, w], BF16, tag="r_tb", name="r_tb")
            rot = pA.tile([128, w], BF16, tag="r_rot", name="r_rot")
            nc.vector.tensor_mul(ta[:], ps[:], cs1_ap)
            nc.vector.tensor_mul(tb[:], ps[:], cs2_ap)
            ta3 = ta[:].rearrange("p (h two d) -> p h two d", two=2, d=32)
            tb3 = tb[:].rearrange("p (h two d) -> p h two d", two=2, d=32)
            r3 = rot[:].rearrange("p (h two d) -> p h two d", two=2, d=32)
            nc.vector.tensor_add(r3[:, :, 0, :], ta3[:, :, 0, :], ta3[:, :, 1, :])
            nc.vector.tensor_sub(r3[:, :, 1, :], tb3[:, :, 1, :], tb3[:, :, 0, :])
            sqt = pA.tile([128, w], BF16, tag="r_sq", name="r_sq")
            nc.vector.tensor_mul(sqt[:], rot[:], rot[:])
            ms = pA.tile([128, nh], F32, tag="r_ms", name="r_ms")
            nc.vector.tensor_reduce(ms[:], sqt[:].rearrange("p (h d) -> p h d", d=64),
                                    axis=mybir.AxisListType.X, op=mybir.AluOpType.add)
            rr = pA.tile([128, nh], F32, tag="r_rr", name="r_rr")
            nc.scalar.activation(rr[:], ms[:], AF.Sqrt, bias=eps_ap, scale=sqrt_scale)
            nc.vector.reciprocal(rr[:], rr[:])
            rrb = rr[:][:, :, None].broadcast_to([128, nh, 64])
            nc.vector.tensor_mul(dst.rearrange("p (h d) -> p h d", d=64),
                                 rot[:].rearrange("p (h d) -> p h d", d=64), rrb)

        # staggered pipeline: projections for tile t, transposes for t-1
        kh_tiles = {}
        qh_tiles = {}

        def emit_proj(t):
            tsl = slice(t * 128, (t + 1) * 128)
            kp = psK.tile([128, HKV * D], F32, tag="kp", name="kp")
            vp = psK.tile([128, HKV * D], F32, tag="vp", name="vp")
            for c in range(ECH):
                nc.tensor.matmul(kp[:], xT[c][:, tsl], WkTs[c][:],
                                 start=(c == 0), stop=(c == ECH - 1))
                nc.tensor.matmul(vp[:], xT[c][:, tsl], WvTs[c][:],
                                 start=(c == 0), stop=(c == ECH - 1))
            kh = pA.tile([128, HKV * D], BF16, tag="kh", name="kh", bufs=3)
            rope_rms(kp, HKV,
                     csk1[:, t * HKV * 64:(t + 1) * HKV * 64],
                     csk2[:, t * HKV * 64:(t + 1) * HKV * 64],
                     kh[:], epsb[:], 1.0 / 64.0)
            kh_tiles[t] = kh
            # v_ext build: (ve * gate) + v  into 65-strided bf16 + valid col
            vet = pA.tile([128, HKV * D], BF16, tag="vet", name="vet")
            nc.sync.dma_start(vet[:], dr["ve"][tsl, :])
            g3 = gate[:, t * HKV:(t + 1) * HKV][:, :, None].broadcast_to([128, HKV, 64])
            vx3 = vex[t][:].rearrange("p (h d) -> p h d", d=65)
            nc.vector.tensor_mul(vx3[:, :, 0:64],
                                 vet[:].rearrange("p (h d) -> p h d", d=64), g3)
            nc.vector.tensor_add(vx3[:, :, 0:64], vx3[:, :, 0:64],
                                 vp[:].rearrange("p (h d) -> p h d", d=64))
            v3 = validT[:, t:t + 1][:, :, None].broadcast_to([128, HKV, 1])
            nc.vector.tensor_copy(vx3[:, :, 64:65], v3)
            if t >= 8:
                tq = t - 8
                qp = psQ.tile([128, H * D], F32, tag="qp", name="qp")
                for c in range(ECH):
                    nc.tensor.matmul(qp[:, 0:512], xT[c][:, tsl], WqTs[c][:, 0:512],
                                     start=(c == 0), stop=(c == ECH - 1))
                    nc.tensor.matmul(qp[:, 512:1024], xT[c][:, tsl],
                                     WqTs[c][:, 512:1024],
                                     start=(c == 0), stop=(c == ECH - 1))
                qh = pA.tile([128, H * D], BF16, tag="qh", name="qh", bufs=3)
                rope_rms(qp, H,
                         csq1[:, tq * H * 64:(tq + 1) * H * 64],
                         csq2[:, tq * H * 64:(tq + 1) * H * 64],
                         qh[:], epsbq[:], 1024.0)
                qh_tiles[tq] = qh

        def emit_transposes(t):
            tsl = slice(t * 128, (t + 1) * 128)
            kh = kh_tiles.pop(t)
            for p in range(0, HKV * D // 128, 2):
                tp = psT.tile([128, 256], BF16, tag="tp", name="tp")
                nc.tensor.transpose(tp[:, 0:128], kh[:, p * 128:(p + 1) * 128], ident[:])
                nc.tensor.transpose(tp[:, 128:256], kh[:, (p + 1) * 128:(p + 2) * 128], ident[:])
                nc.scalar.copy(kTt[p][:, tsl], tp[:, 0:128])
                nc.scalar.copy(kTt[p + 1][:, tsl], tp[:, 128:256])
            if t >= 8:
                tq = t - 8
                qh = qh_tiles.pop(tq)
                qsl = slice(tq * 128, (tq + 1) * 128)
                for p in range(0, H * D // 128, 2):
                    tp = psT.tile([128, 256], BF16, tag="tp", name="tp")
                    nc.tensor.transpose(tp[:, 0:128], qh[:, p * 128:(p + 1) * 128], ident[:])
                    nc.tensor.transpose(tp[:, 128:256], qh[:, (p + 1) * 128:(p + 2) * 128], ident[:])
                    nc.scalar.copy(qTt[p][:, qsl], tp[:, 0:128])
                    nc.scalar.copy(qTt[p + 1][:, qsl], tp[:, 128:256])

        for t in range(NTT):
            emit_proj(t)
            if t >= 2:
                emit_transposes(t - 2)
        emit_transposes(NTT - 2)
        emit_transposes(NTT - 1)

        phA.close()

        # ---- phase B: attention per head pair ----
        pB = ctx.enter_context(tc.tile_pool(name="pB", bufs=VSTAG + 1))
        pBs = ctx.enter_context(tc.tile_pool(name="pBs", bufs=2))
        psB = ctx.enter_context(contextlib.ExitStack())
        psS = psB.enter_context(tc.tile_pool(name="psS", bufs=2, space="PSUM"))
        psY = psB.enter_context(tc.tile_pool(name="psY", bufs=2, space="PSUM"))

        for pi in range(H // 2):
            posA, posB = 2 * pi, 2 * pi + 1
            hA, hB = PERM[posA], PERM[posB]
            kvA, kvB = hA // 2, hB // 2
            ktile = kTt[pi // 2]
            qtile = qTt[pi]
            ypAB = psY.tile([128, 2 * OWN], F32, tag="ypAB", name="ypAB")
            ypA = ypAB[:, 0:OWN]
            ypB = ypAB[:, OWN:2 * OWN]
            eTs = {}

            def emit_scores(j):
                jsl = slice(j * 128, (j + 1) * 128)
                sp = psS.tile([128, 2 * OWN], F32, tag="sp", name="sp")
                # two row-group-concurrent K=64 matmuls (tile_position
                # auto-derives (0,0) / (64,0) from the base partitions)
                nc.tensor.matmul(sp[:, 0:OWN], ktile[0:64, jsl],
                                 qtile[0:64, :], start=True, stop=True)
                nc.tensor.matmul(sp[:, OWN:2 * OWN], ktile[64:128, jsl],
                                 qtile[64:128, :], start=True, stop=True)
                eT = pB.tile([128, 2 * OWN], BF16, tag="eT", name="eT",
                             bufs=VSTAG + 1)
                if j in DVE_EXP_J:
                    nc.vector.tensor_scalar(
                        out=eT[:].bitcast(mybir.dt.int16), in0=sp[:],
                        scalar1=A_SCH, scalar2=B_SCH,
                        op0=mybir.AluOpType.mult, op1=mybir.AluOpType.add)
                else:
                    nc.scalar.activation(eT[:], sp[:], AF.Exp, scale=32.0)
                if j in MASK_IDX:
                    m = MASK_IDX[j]
                    msl = maskT[:, m * 2 * OWN:(m + 1) * 2 * OWN]
                    if j in GPS_MASK_J:
                        nc.gpsimd.tensor_tensor(eT[:], eT[:], msl,
                                                op=mybir.AluOpType.mult)
                    else:
                        nc.vector.tensor_mul(eT[:], eT[:], msl)
                eTs[j] = eT

            def emit_values(j):
                eT = eTs.pop(j)
                nc.tensor.matmul(ypA[0:65, :], vex[j][:, kvA * 65:(kvA + 1) * 65],
                                 eT[:, 0:OWN], start=(j == 0), stop=(j == NTT - 1))
                nc.tensor.matmul(ypB[0:65, :], vex[j][:, kvB * 65:(kvB + 1) * 65],
                                 eT[:, OWN:2 * OWN], start=(j == 0), stop=(j == NTT - 1))

            for j in range(NTT):
                emit_scores(j)
                if j >= VSTAG:
                    emit_values(j - VSTAG)
            for j in range(NTT - VSTAG, NTT):
                emit_values(j)

            # normalization: DMA-reshape the den row [1, 2*OWN] to [8,128]
            # (8 DVE lanes instead of 1), bit-trick reciprocal + 1 Newton
            # there, reshape back, then DMA log-doubling broadcast.
            den = ypAB[64:65, :]
            den_sb = pBs.tile([1, 2 * OWN], F32, tag="den_sb", name="den_sb")
            nc.scalar.copy(den_sb[:], den)
            dsb8 = pBs.tile([8, 128], F32, tag="dsb8", name="dsb8")
            nc.sync.dma_start(dsb8[:], den_sb[:].rearrange("o (r c) -> o r c", r=8))
            y0i = pBs.tile([8, 128], mybir.dt.int32, tag="y0i", name="y0i")
            nc.vector.tensor_tensor(y0i[:], magicT[:], dsb8[:].bitcast(mybir.dt.int32),
                                    op=mybir.AluOpType.subtract)
            tt = pBs.tile([8, 128], F32, tag="tt", name="tt")
            nc.vector.tensor_mul(tt[:], dsb8[:], y0i[:].bitcast(F32))
            nc.vector.tensor_scalar(out=tt[:], in0=tt[:], scalar1=-1.0,
                                    scalar2=2.0, op0=mybir.AluOpType.mult,
                                    op1=mybir.AluOpType.add)
            rec8 = pBs.tile([8, 128], F32, tag="rec8", name="rec8")
            nc.vector.tensor_mul(rec8[:], y0i[:].bitcast(F32), tt[:])
            bcA = pBs.tile([64, OWN], F32, tag="bcA", name="bcA")
            bcB = pBs.tile([64, OWN], F32, tag="bcB", name="bcB")
            nc.sync.dma_start(bcA[0:1, :].rearrange("o (r c) -> o r c", r=4), rec8[0:4, :])
            nc.sync.dma_start(bcB[0:1, :].rearrange("o (r c) -> o r c", r=4), rec8[4:8, :])
            w = 1
            while w < 64:
                nc.sync.dma_start(bcA[w:2 * w, :], bcA[0:w, :])
                nc.sync.dma_start(bcB[w:2 * w, :], bcB[0:w, :])
                w *= 2
            dst = yT[pi]
            nc.vector.tensor_mul(dst[0:64, :], ypA[0:64, :], bcA[:])
            ytmp = pBs.tile([64, OWN], BF16, tag="ytmp", name="ytmp")
            nc.vector.tensor_mul(ytmp[:], ypB[0:64, :], bcB[:])
            nc.sync.dma_start(dst[64:128, :], ytmp[:])

        psB.close()

        # ---- phase C: output projection (token-major) ----
        psO = ctx.enter_context(tc.tile_pool(name="psO", bufs=2, space="PSUM"))
        pC = ctx.enter_context(tc.tile_pool(name="pC", bufs=2))
        for m in range(NQT):
            msl = slice(m * 128, (m + 1) * 128)
            for eh in range(2):
                esl = slice(eh * 512, (eh + 1) * 512)
                op = psO.tile([128, 512], F32, tag="op", name="op")
                for c in range(ECH):
                    nc.tensor.matmul(op[:], yT[c][:, msl], WpTs[c][:, esl],
                                     start=(c == 0), stop=(c == ECH - 1))
                ot = pC.tile([128, 512], F32, tag="ot", name="ot")
                nc.vector.tensor_copy(ot[:], op[:])
                nc.sync.dma_start(dr["out"][msl, esl], ot[:])


# ---------------------------------------------------------------------------
# host side
# ---------------------------------------------------------------------------

_CACHE = {}


def _host_prep(x, ve, cos, sin, Wq, Wk, Wv, Wproj, Wgate):
    bf = ml_dtypes.bfloat16
    cos2 = np.asarray(cos, np.float32).reshape(T, 32)
    sin2 = np.asarray(sin, np.float32).reshape(T, 32)
    shared = {
        "WqT": np.ascontiguousarray(
            np.asarray(Wq, np.float32).T.reshape(E, H, D)[:, PERM, :]
            .reshape(E, H * D)).astype(bf),
        "WkT": np.ascontiguousarray(np.asarray(Wk, np.float32).T).astype(bf),
        "WvT": np.ascontiguousarray(np.asarray(Wv, np.float32).T).astype(bf),
        "WgT": np.ascontiguousarray(np.asarray(Wgate, np.float32).T).astype(bf),
        "WpT": np.ascontiguousarray(
            np.asarray(Wproj, np.float32).T.reshape(H, D, E)[PERM]
            .reshape(E, E)).astype(bf),
    }
    # band masks, scoresT layout, duplicated for the head pair, shared by
    # all cores
    maskT = np.zeros((8, 128, OWN), np.float32)
    for m, j in enumerate(MASKED_J):
        c = j * 128 + np.arange(128)[:, None]
        q = np.arange(OWN)[None, :]
        maskT[m] = ((c >= q + 1) & (c <= q + WIN)).astype(np.float32)
    shared["maskT"] = np.concatenate([maskT, maskT], axis=2).astype(bf)
    in_maps = []
    for core in range(NCORES):
        b, s = core // 4, core % 4
        base = s * 512
        lo = base - WIN
        n0 = max(0, -lo)
        xh = np.zeros((KT, E), np.float32)
        veh = np.zeros((KT, HKV * D), np.float32)
        ck = np.zeros((KT, 32), np.float32)
        sk = np.zeros((KT, 32), np.float32)
        xh[n0:] = x[b, lo + n0:base + OWN]
        veh[n0:] = ve[b, lo + n0:base + OWN]
        ck[n0:] = cos2[lo + n0:base + OWN]
        sk[n0:] = sin2[lo + n0:base + OWN]
        valid = np.zeros((KT,), np.float32)
        valid[n0:] = 1.0
        csk = np.concatenate([ck, sk], axis=1)       # [KT, 64] = [cos|sin]
        ssk = np.concatenate([sk, ck], axis=1)       # [KT, 64] = [sin|cos]
        m = {
            "xT": np.ascontiguousarray(xh.T).astype(bf),
            "ve": veh.astype(bf),
            "csk1": np.tile(csk, (1, HKV)).astype(bf),
            "csk2": np.tile(ssk, (1, HKV)).astype(bf),
            "csq1": np.tile(csk[WIN:], (1, H)).astype(bf),
            "csq2": np.tile(ssk[WIN:], (1, H)).astype(bf),
            "validT": np.ascontiguousarray(
                valid.reshape(NTT, 128).T).astype(bf),
        }
        m.update(shared)
        in_maps.append(m)
    return in_maps


def _get_nc():
    if "nc" not in _CACHE:
        _CACHE["nc"] = _build_nc()
    return _CACHE["nc"]


def kernel(x, ve, cos, sin, Wq, Wk, Wv, Wproj, Wgate, window_size=WIN,
           _trace=False):
    assert int(window_size) == WIN
    in_maps = _host_prep(x, ve, cos, sin, Wq, Wk, Wv, Wproj, Wgate)
    nc = _get_nc()
    res = run_bass_kernel_spmd(nc, in_maps, core_ids=list(range(NCORES)),
                               trace=_trace)
    _CACHE["last_result"] = res
    out = np.zeros((B, T, E), np.float32)
    for core in range(NCORES):
        b, s = core // 4, core % 4
        out[b, s * 512:(s + 1) * 512] = res.results[core]["out"]
    return out
